# revision 29
# baseline (speedup 1.0000x reference)
"""Trainium2 Bass kernel for nn_ASTGATClassifier (3-layer GAT + BN + ELU + pool + MLP).

Strategy (8 NeuronCores, SPMD single program):
  - Nodes/dst sharded contiguously: core c owns nodes [6250c, 6250(c+1)).
  - Edges (incl. self-loops, E'=450K) sorted by dst, owned by dst core.
  - Per layer: x = h @ W^T computed sharded (transposed layout hT as lhsT),
    attention alphas folded per-edge, packed rows [x | al_s] all-gathered into
    each core's HBM (bf16), then per dst-block (128 dst nodes) the edge lists
    are dma_gather'ed and scatter-added via one-hot matmuls into PSUM.
  - Softmax denominators accumulated with the same one-hot matmuls; division
    folded into the PSUM flush (no max-subtraction needed: |logit| <~ 15).
  - BatchNorm: col sums/sumsq via ones-matmul + AllReduce; affine+ELU applied
    in transposed layout (per-partition scale/bias) which also produces the
    lhsT for the next layer's matmul.
  - Pooling: per-graph segment reduce (sum/max) over the transposed final
    features; every core runs all cores' segment programs (SPMD uniformity)
    and masks out the ones that aren't its own; AllReduce(add/max) combines.
  - Classifier replicated; output from core 0.

All graph-dependent structure (tile counts, segment ranges) is baked into the
program at trace time from the *inputs*, made uniform across cores by padding
to per-block maxima.
"""

import sys

sys.path.insert(0, "/opt/trn_rl_repo")

import numpy as np
import ml_dtypes

N_NODES = 50000
N_EDGES = 400000
N_GRAPHS = 256
NUM_TYPES = 200
EMB = 64
HID = 128
HEADS = 4
GDIM = 256
NUM_CLASSES = 20
EPS = 1e-5
NEG = 0.2

NC = 8
NSH = N_NODES // NC          # 6250 nodes per core
NBLK = (NSH + 127) // 128    # 49 blocks
N2 = NBLK * 128              # 6272 padded shard
NTOT = NC * N2               # 50176 padded table rows
HALF = (NC // 2) * N2        # 25088 row split for int16 gather indices

BF16 = ml_dtypes.bfloat16

_CACHE = {}


def _wrap_idx(idx):
    """int16 gather index layout: [128, n/16]; idx j at [j%16, j//16], tiled x8."""
    n = len(idx)
    assert n % 16 == 0
    a = np.asarray(idx, dtype=np.int16).reshape(n // 16, 16).T  # [16, n/16]
    return np.tile(a, (8, 1))  # [128, n/16]


def _pad128(lst, padval):
    n = len(lst)
    r = (-n) % 128
    if r:
        lst = np.concatenate([lst, np.full(r, padval, dtype=np.int64)])
    return lst


def preprocess(x, edge_index, depth, batch):
    """Host-side index preprocessing -> per-core blobs + uniform schedule."""
    x = np.asarray(x).astype(np.int64)
    ei = np.asarray(edge_index).astype(np.int64)
    batch = np.asarray(batch).astype(np.int64)
    loop = np.arange(N_NODES, dtype=np.int64)
    src = np.concatenate([ei[0], loop])
    dst = np.concatenate([ei[1], loop])
    order = np.argsort(dst, kind="stable")
    src, dst = src[order], dst[order]

    core_of = np.minimum(src // NSH, NC - 1)
    row = core_of * N2 + (src - core_of * NSH)  # padded table row of src

    # per (core, block): lo/hi bucket edge lists
    Tlo = np.zeros((NC, NBLK), dtype=np.int64)
    Thi = np.zeros((NC, NBLK), dtype=np.int64)
    buckets = {}
    for c in range(NC):
        lo_n, hi_n = c * NSH, (c + 1) * NSH
        m = (dst >= lo_n) & (dst < hi_n)
        s_c, d_c, r_c = src[m], dst[m] - lo_n, row[m]
        for b in range(NBLK):
            mb = (d_c >= b * 128) & (d_c < min((b + 1) * 128, NSH))
            rb, db = r_c[mb], d_c[mb] - b * 128
            lo = rb < HALF
            buckets[(c, b)] = (rb[lo], db[lo], rb[~lo] - HALF, db[~lo])
            Tlo[c, b] = (len(rb[lo]) + 127) // 128
            Thi[c, b] = (len(rb[~lo]) + 127) // 128
    TloU = Tlo.max(axis=0)  # uniform schedule across cores
    ThiU = Thi.max(axis=0)

    # build per-core blobs
    blobs = []
    for c in range(NC):
        ilo_cols, ihi_cols, iald_cols, dstloc_cols = [], [], [], []
        for b in range(NBLK):
            rlo, dlo, rhi, dhi = buckets[(c, b)]
            nlo, nhi = TloU[b] * 128, ThiU[b] * 128
            rlo_p = np.concatenate([rlo, np.zeros(nlo - len(rlo), dtype=np.int64)])
            rhi_p = np.concatenate([rhi, np.zeros(nhi - len(rhi), dtype=np.int64)])
            dlo_p = np.concatenate([dlo, np.full(nlo - len(dlo), -1, dtype=np.int64)])
            dhi_p = np.concatenate([dhi, np.full(nhi - len(dhi), -1, dtype=np.int64)])
            if nlo:
                ilo_cols.append(_wrap_idx(rlo_p))
            if nhi:
                ihi_cols.append(_wrap_idx(rhi_p))
            dall = np.concatenate([dlo_p, dhi_p])
            # ald gather index: local dst (padded rows use 0)
            iald_cols.append(_wrap_idx(np.where(dall >= 0, dall + b * 128, 0)))
            dstloc_cols.append(dall.reshape(-1, 128).T.astype(np.float32))
        blobs.append(
            dict(
                idx_lo=np.concatenate(ilo_cols, axis=1) if ilo_cols else np.zeros((128, 8), np.int16),
                idx_hi=np.concatenate(ihi_cols, axis=1) if ihi_cols else np.zeros((128, 8), np.int16),
                idx_ald=np.concatenate(iald_cols, axis=1),
                dstloc=np.concatenate(dstloc_cols, axis=1),
            )
        )

    # emb gather idx + depth rows per core
    for c in range(NC):
        ids = np.zeros(N2, dtype=np.int64)
        ids[:NSH] = x[c * NSH : (c + 1) * NSH]
        blobs[c]["emb_idx"] = _wrap_idx(ids)
        dr = np.zeros((1, N2), dtype=np.float32)
        dr[0, :NSH] = np.asarray(depth, dtype=np.float32)[c * NSH : (c + 1) * NSH]
        blobs[c]["depth_row"] = dr

    # pooling segments: graph runs in sorted batch
    counts = np.bincount(batch, minlength=N_GRAPHS)
    starts = np.concatenate([[0], np.cumsum(counts)])
    segs = []  # list per core-cc of (col_a, col_b, g, inv_cnt)
    for cc in range(NC):
        lo_n, hi_n = cc * NSH, (cc + 1) * NSH
        lst = []
        for g in range(N_GRAPHS):
            a, bnd = starts[g], starts[g + 1]
            aa, bb = max(a, lo_n), min(bnd, hi_n)
            if aa < bb:
                lst.append((int(aa - lo_n), int(bb - lo_n), int(g), float(1.0 / max(counts[g], 1))))
        segs.append(lst)
    for c in range(NC):
        m01 = np.zeros((128, N_GRAPHS), dtype=np.float32)
        gs_here = set(g for (_, _, g, _) in segs[c])
        for g in gs_here:
            m01[:, g] = 1.0
        m8 = np.zeros((128, NC), dtype=np.float32)
        m8[:, c] = 1.0
        m8n = np.where(m8 > 0, 0.0, -1e30).astype(np.float32)
        blobs[c]["mask8"] = m8
        blobs[c]["mask8n"] = m8n

    return dict(TloU=TloU, ThiU=ThiU, blobs=blobs, segs=segs)


def build_param_blobs(p):
    """Host-side parameter layout transforms (bf16 casts, folds, transposes)."""
    f32 = np.float32
    out = {}

    def fold_a(W, a_s, a_d, heads, c):
        # al[n,h] = sum_c x[n,h,c] a[h,c];  x = h @ W.T
        # -> al = h @ A with A[k,h] = sum_c W[h*c + c, k] * a[h,c]
        W3 = W.reshape(heads, c, -1)  # [H, C, IN]
        As = np.einsum("hck,hc->kh", W3, a_s).astype(f32)
        Ad = np.einsum("hck,hc->kh", W3, a_d).astype(f32)
        return np.concatenate([As, Ad], axis=1)  # [IN, 2H]

    out["w0x"] = np.ascontiguousarray(p["W0"].T).astype(BF16)          # [64, 512]
    out["w0al"] = fold_a(p["W0"], p["as0"], p["ad0"], HEADS, HID).astype(BF16)  # [64, 8]
    out["w1x"] = np.ascontiguousarray(p["W1"].T).astype(BF16)          # [512, 512]
    out["w1al"] = fold_a(p["W1"], p["as1"], p["ad1"], HEADS, HID).astype(BF16)
    out["w2x"] = np.ascontiguousarray(p["W2"].T).astype(BF16)          # [512, 128]
    out["w2al"] = fold_a(p["W2"], p["as2"], p["ad2"], 1, GDIM // 2).astype(BF16)  # [512, 2]
    out["emb_t"] = np.asarray(p["emb_table"], dtype=f32)               # [200, 64]
    out["dw_row"] = np.asarray(p["depth_w"], dtype=f32).reshape(1, EMB)
    out["db_row"] = np.asarray(p["depth_b"], dtype=f32).reshape(1, EMB)
    for l, (g, be) in enumerate([(p["g0"], p["be0"]), (p["g1"], p["be1"]), (p["g2"], p["be2"])]):
        out[f"gam{l}"] = np.asarray(g, dtype=f32).reshape(1, -1)
        out[f"bet{l}"] = np.asarray(be, dtype=f32).reshape(1, -1)
    out["cw1t"] = np.ascontiguousarray(p["cw1"].T).astype(f32)         # [256, 256]
    out["cb1c"] = np.asarray(p["cb1"], dtype=f32).reshape(2, 128).T.copy()  # [128, 2]
    out["cw2t"] = np.ascontiguousarray(p["cw2"].T).astype(f32)         # [256, 20]
    out["cb2c"] = np.asarray(p["cb2"], dtype=f32).reshape(NUM_CLASSES, 1)
    out["iota"] = np.tile(np.arange(128, dtype=f32)[None, :], (128, 1))
    out["iotab"] = out["iota"].astype(BF16)
    out["iden_f"] = np.eye(128, dtype=f32)
    out["iden_b"] = np.eye(128).astype(BF16)
    out["ones_b"] = np.ones((128, 1), dtype=BF16)
    out["ones_r"] = np.ones((1, 128), dtype=f32)
    return out


def build_nc(pre):
    """Trace the full SPMD bass program (structure from `pre`)."""
    import concourse.bacc as bacc
    import concourse.bass as bass
    import concourse.mybir as mybir
    import concourse.tile as tile
    from concourse.library_config import mlp

    dt = mybir.dt
    ALU = mybir.AluOpType
    ACTF = mybir.ActivationFunctionType
    AXX = mybir.AxisListType.X

    TloU, ThiU, segs = pre["TloU"], pre["ThiU"], pre["segs"]
    Tall = TloU + ThiU
    Tmax = int(Tall.max())
    totT = int(Tall.sum())

    nc = bacc.Bacc("TRN2", target_bir_lowering=False, debug=False, num_devices=NC)

    b0 = pre["blobs"][0]
    EIN = {}

    def ein(name, arr_like, dtyp):
        EIN[name] = nc.dram_tensor(name, list(arr_like.shape), dtyp, kind="ExternalInput").ap()
        return EIN[name]

    # per-core data inputs
    i_lo = ein("idx_lo", b0["idx_lo"], dt.int16)
    i_hi = ein("idx_hi", b0["idx_hi"], dt.int16)
    i_ald = ein("idx_ald", b0["idx_ald"], dt.int16)
    i_dstloc = ein("dstloc", b0["dstloc"], dt.float32)
    i_embidx = ein("emb_idx", b0["emb_idx"], dt.int16)
    i_depth = ein("depth_row", b0["depth_row"], dt.float32)
    i_mask8 = ein("mask8", b0["mask8"], dt.float32)
    i_mask8n = ein("mask8n", b0["mask8n"], dt.float32)
    # params
    P = {}
    P["w0x"] = ein("w0x", np.zeros((EMB, HEADS * HID)), dt.bfloat16)
    P["w0al"] = ein("w0al", np.zeros((EMB, 2 * HEADS)), dt.bfloat16)
    P["w1x"] = ein("w1x", np.zeros((HEADS * HID, HEADS * HID)), dt.bfloat16)
    P["w1al"] = ein("w1al", np.zeros((HEADS * HID, 2 * HEADS)), dt.bfloat16)
    P["w2x"] = ein("w2x", np.zeros((HEADS * HID, GDIM // 2)), dt.bfloat16)
    P["w2al"] = ein("w2al", np.zeros((HEADS * HID, 2)), dt.bfloat16)
    P["emb_t"] = ein("emb_t", np.zeros((NUM_TYPES, EMB)), dt.float32)
    P["dw_row"] = ein("dw_row", np.zeros((1, EMB)), dt.float32)
    P["db_row"] = ein("db_row", np.zeros((1, EMB)), dt.float32)
    for l, oc in [(0, 512), (1, 512), (2, 128)]:
        P[f"gam{l}"] = ein(f"gam{l}", np.zeros((1, oc)), dt.float32)
        P[f"bet{l}"] = ein(f"bet{l}", np.zeros((1, oc)), dt.float32)
    P["cw1t"] = ein("cw1t", np.zeros((GDIM, GDIM)), dt.float32)
    P["cb1c"] = ein("cb1c", np.zeros((128, 2)), dt.float32)
    P["cw2t"] = ein("cw2t", np.zeros((GDIM, NUM_CLASSES)), dt.float32)
    P["cb2c"] = ein("cb2c", np.zeros((NUM_CLASSES, 1)), dt.float32)
    P["iota"] = ein("iota", np.zeros((128, 128)), dt.float32)
    P["iotab"] = ein("iotab", np.zeros((128, 128)), dt.bfloat16)
    P["iden_f"] = ein("iden_f", np.zeros((128, 128)), dt.float32)
    P["iden_b"] = ein("iden_b", np.zeros((128, 128)), dt.bfloat16)
    P["ones_b"] = ein("ones_b", np.zeros((128, 1)), dt.bfloat16)
    P["ones_r"] = ein("ones_r", np.zeros((1, 128)), dt.float32)

    out_dram = nc.dram_tensor("out", [N_GRAPHS, NUM_CLASSES], dt.float32, kind="ExternalOutput").ap()

    LCFG = [  # (IN, OC, H, EW, wx, wal)
        (EMB, 512, 4, 640, "w0x", "w0al"),
        (512, 512, 4, 640, "w1x", "w1al"),
        (512, 128, 1, 256, "w2x", "w2al"),
    ]
    EWMAX = 640

    from contextlib import ExitStack

    import os as _os2
    _SKIP2 = set(_os2.environ.get("KSKIP2", "").split(","))
    with tile.TileContext(nc) as tc, ExitStack() as stk:
        nc.gpsimd.load_library(mlp)
        sb = stk.enter_context(tc.tile_pool(name="sb", bufs=2))
        sb1 = stk.enter_context(tc.tile_pool(name="sb1", bufs=1))
        ps = stk.enter_context(tc.tile_pool(name="ps", bufs=2, space="PSUM"))
        dram = stk.enter_context(tc.tile_pool(name="dram", bufs=1, space="DRAM"))

        # ---- resident constants / params in SBUF
        def load_sb(ap, shape, dtyp, tag, pool=sb1):
            t = pool.tile(shape, dtyp, tag=tag)
            nc.sync.dma_start(t[:], ap[:, :])
            return t

        iota_sb = load_sb(P["iotab"], [128, 128], dt.bfloat16, "iota")
        idenb_sb = load_sb(P["iden_b"], [128, 128], dt.bfloat16, "idenb")
        idenf_sb = load_sb(P["iden_f"], [128, 128], dt.float32, "idenf")
        onesb_sb = load_sb(P["ones_b"], [128, 1], dt.bfloat16, "onesb")
        onesr_sb = load_sb(P["ones_r"], [1, 128], dt.float32, "onesr")
        dstloc_sb = load_sb(i_dstloc, [128, totT], dt.float32, "dstloc")
        dw_sb = load_sb(P["dw_row"], [1, EMB], dt.float32, "dwrow")
        db_sb = load_sb(P["db_row"], [1, EMB], dt.float32, "dbrow")
        wx_sb = {}
        wal_sb = {}
        for l, (IN, OC, H, EW, wx, wal) in enumerate(LCFG):
            nch_in = (IN + 127) // 128
            wx_sb[l] = []
            wal_sb[l] = []
            for k in range(nch_in):
                kp = min(IN - k * 128, 128)
                tx = sb1.tile([kp, OC], dt.bfloat16, tag=f"wx{l}_{k}")
                nc.sync.dma_start(tx[:], P[wx][k * 128 : k * 128 + kp, :])
                wx_sb[l].append(tx)
                ta = sb1.tile([kp, 2 * H], dt.bfloat16, tag=f"wal{l}_{k}")
                nc.sync.dma_start(ta[:], P[wal][k * 128 : k * 128 + kp, :])
                wal_sb[l].append(ta)

        # big persistent SBUF buffers
        h_raw = sb1.tile([128, NBLK * 512], dt.bfloat16, tag="h_raw")
        hTe = sb1.tile([128, 4 * N2], dt.bfloat16, tag="hTe")  # transposed ELU'd features

        # DRAM tiles
        ald_dram = dram.tile([N2, 128], dt.bfloat16, tag="ald")
        xa_fulls = {}

        # =========================================================
        # Layer-0 node phase: hT0 (transposed input features, bf16)
        # =========================================================
        embidx_sb = load_sb(i_embidx, [128, N2 // 16], dt.int16, "embidx")
        emb_g = sb.tile([128, NBLK * EMB], dt.float32, tag="xg", name="emb_g")
        nc.gpsimd.dma_gather(
            emb_g[:].rearrange("p (t w) -> p t w", w=EMB),
            P["emb_t"][:, :],
            embidx_sb[:],
            N2, N2, EMB,
        )
        for nt in range(NBLK):
            dr_t = sb.tile([1, 128], dt.float32, tag="dr", bufs=2)
            nc.sync.dma_start(dr_t[:], i_depth[0:1, nt * 128 : (nt + 1) * 128])
            ps_t = ps.tile([EMB, 128], dt.float32, tag="med")
            nc.tensor.matmul(out=ps_t[:], lhsT=dw_sb[:], rhs=dr_t[:], start=True, stop=False)
            nc.tensor.matmul(out=ps_t[:], lhsT=db_sb[:], rhs=onesr_sb[:], start=False, stop=False)
            nc.tensor.matmul(
                out=ps_t[:],
                lhsT=emb_g[:, nt * EMB : (nt + 1) * EMB],
                rhs=idenf_sb[:],
                is_transpose=True,
                start=False,
                stop=True,
            )
            nc.vector.tensor_copy(hTe[0:EMB, nt * 128 : (nt + 1) * 128], ps_t[:])

        # =========================================================
        # generic phases
        # =========================================================
        def x_phase(l, lhsT_tile, lhsT_p0, nchunks):
            IN, OC, H, EW, _, _ = LCFG[l]
            xa_in = dram.tile([N2, EW], dt.bfloat16, tag="xa_in")
            xa_full = dram.tile([NTOT, EW], dt.bfloat16, tag="xa_full", addr_space="Shared")
            xa_fulls[l] = xa_full
            for nt in range(NBLK):
                ps_x = ps.tile([128, OC], dt.float32, tag="big")
                ps_al = ps.tile([128, 2 * H], dt.float32, tag="small")
                for k in range(nchunks):
                    lhs = lhsT_tile[0 : min(IN, 128), k * N2 + nt * 128 : k * N2 + (nt + 1) * 128] if nchunks > 1 else lhsT_tile[0:IN, nt * 128 : (nt + 1) * 128]
                    nc.tensor.matmul(out=ps_x[:], lhsT=lhs, rhs=wx_sb[l][k][:], start=(k == 0), stop=(k == nchunks - 1))
                    nc.tensor.matmul(out=ps_al[:], lhsT=lhs, rhs=wal_sb[l][k][:], start=(k == 0), stop=(k == nchunks - 1))
                xa_t = sb.tile([128, EW], dt.bfloat16, tag="xa_t")
                nc.vector.tensor_copy(xa_t[:, 0:OC], ps_x[:])
                nc.vector.tensor_copy(xa_t[:, OC : OC + H], ps_al[:, 0:H])
                ald_t = sb.tile([128, 128], dt.bfloat16, tag="ald_t")
                nc.vector.tensor_copy(ald_t[:, 0:H], ps_al[:, H : 2 * H])
                nc.sync.dma_start(xa_in[nt * 128 : (nt + 1) * 128, :], xa_t[:])
                nc.sync.dma_start(ald_dram[nt * 128 : (nt + 1) * 128, :], ald_t[:])
            nc.gpsimd.collective_compute(
                "AllGather",
                ALU.bypass,
                ins=[xa_in[:, :]],
                outs=[xa_full[:, :]],
                replica_groups=[list(range(NC))],
            )

        def edge_phase(l):
            IN, OC, H, EW, _, _ = LCFG[l]
            xa_full = xa_fulls[l]
            Cw = OC // H
            nc.vector.memset(h_raw[:, 0 : NBLK * OC], 0)
            colT = 0  # running tile index
            col_lo = 0
            col_hi = 0
            for b in range(NBLK):
                tlo, thi = int(TloU[b]), int(ThiU[b])
                T = tlo + thi
                vd = 128 if b < NBLK - 1 else NSH - (NBLK - 1) * 128
                xg = sb.tile([128, Tmax * EWMAX], dt.bfloat16, tag="xg")
                xg3 = xg[:, 0 : T * EW].rearrange("p (t w) -> p t w", w=EW)
                if tlo:
                    ilo_t = sb.tile([128, tlo * 8], dt.int16, tag="ilo")
                    nc.sync.dma_start(ilo_t[:], i_lo[:, col_lo : col_lo + tlo * 8])
                    nc.gpsimd.dma_gather(
                        xg3[:, 0:tlo, :], xa_full[0:HALF, :], ilo_t[:], tlo * 128, tlo * 128, EW
                    )
                if thi:
                    ihi_t = sb.tile([128, thi * 8], dt.int16, tag="ihi")
                    nc.sync.dma_start(ihi_t[:], i_hi[:, col_hi : col_hi + thi * 8])
                    nc.gpsimd.dma_gather(
                        xg3[:, tlo:T, :], xa_full[HALF:NTOT, :], ihi_t[:], thi * 128, thi * 128, EW
                    )
                _ = None
                iald_t = sb.tile([128, Tmax * 8], dt.int16, tag="iald")
                nc.sync.dma_start(iald_t[:, 0 : T * 8], i_ald[:, colT * 8 : (colT + T) * 8])
                aldg = sb.tile([128, Tmax * 128], dt.bfloat16, tag="aldg")
                aldg3 = aldg[:, 0 : T * 128].rearrange("p (t w) -> p t w", w=128)
                nc.gpsimd.dma_gather(
                    aldg3, ald_dram[:, :], iald_t[:, 0 : T * 8], T * 128, T * 128, 128,
                )
                # logits -> exp
                zt = sb.tile([128, Tmax * H], dt.float32, tag="zt")
                nc.vector.tensor_tensor(
                    out=zt[:, 0 : T * H],
                    in0=xg3[:, 0:T, OC : OC + H],
                    in1=aldg3[:, 0:T, 0:H],
                    op=ALU.add,
                )
                z2 = sb.tile([128, Tmax * H], dt.float32, tag="z2")
                nc.vector.tensor_scalar(out=z2[:, 0 : T * H], in0=zt[:, 0 : T * H], scalar1=NEG, scalar2=None, op0=ALU.mult)
                nc.vector.tensor_tensor(out=z2[:, 0 : T * H], in0=zt[:, 0 : T * H], in1=z2[:, 0 : T * H], op=ALU.max)
                ex = sb.tile([128, Tmax * H], dt.float32, tag="ex")
                nc.scalar.activation(ex[:, 0 : T * H], z2[:, 0 : T * H], ACTF.Exp)
                exb = sb.tile([128, Tmax * H], dt.bfloat16, tag="exb")
                nc.vector.tensor_copy(exb[:, 0 : T * H], ex[:, 0 : T * H])
                ps_o = ps.tile([128, OC], dt.float32, tag="big")
                ps_d = ps.tile([128, max(H, 2)], dt.float32, tag="small")
                for t in range(T):
                    oh = sb.tile([128, 128], dt.bfloat16, tag="oh")
                    if "nooh" not in _SKIP2:
                        nc.vector.tensor_scalar(
                            out=oh[:], in0=iota_sb[:], scalar1=dstloc_sb[:, colT + t : colT + t + 1],
                            scalar2=None, op0=ALU.is_equal,
                        )
                    if "noscale" in _SKIP2:
                        xgs_ap = xg3[:, t, 0:OC]
                    else:
                        xgs = sb.tile([128, OC], dt.bfloat16, tag="xgs")
                        for h in range(H):
                            nc.vector.tensor_scalar(
                                out=xgs[:, h * Cw : (h + 1) * Cw],
                                in0=xg3[:, t, h * Cw : (h + 1) * Cw],
                                scalar1=ex[:, t * H + h : t * H + h + 1],
                                scalar2=None, op0=ALU.mult,
                            )
                        xgs_ap = xgs[:]
                    nc.tensor.matmul(out=ps_o[:], lhsT=oh[:], rhs=xgs_ap, start=(t == 0), stop=(t == T - 1))
                    nc.tensor.matmul(out=ps_d[:, 0:H], lhsT=oh[:], rhs=exb[:, t * H : (t + 1) * H], start=(t == 0), stop=(t == T - 1))
                rd = sb.tile([128, max(H, 2)], dt.float32, tag="rd")
                nc.vector.reciprocal(rd[0:vd, 0:H], ps_d[0:vd, 0:H])
                for h in range(H):
                    nc.vector.tensor_scalar(
                        out=h_raw[0:vd, b * OC + h * Cw : b * OC + (h + 1) * Cw],
                        in0=ps_o[0:vd, h * Cw : (h + 1) * Cw],
                        scalar1=rd[0:vd, h : h + 1],
                        scalar2=None, op0=ALU.mult,
                    )
                colT += T
                col_lo += tlo * 8
                col_hi += thi * 8

        def bn_elu_phase(l):
            """stats -> AllReduce -> coeffs -> transpose+BN+ELU -> hTe chunks."""
            IN, OC, H, EW, _, _ = LCFG[l]
            nch = OC // 128
            gam_t = sb.tile([1, OC], dt.float32, tag="gamt", bufs=1)
            nc.sync.dma_start(gam_t[:], P[f"gam{l}"][0:1, 0:OC])
            bet_t = sb.tile([1, OC], dt.float32, tag="bett", bufs=1)
            nc.sync.dma_start(bet_t[:], P[f"bet{l}"][0:1, 0:OC])
            ps_s = ps.tile([1, OC], dt.float32, tag="row")
            ps_q = ps.tile([1, OC], dt.float32, tag="row")
            for b in range(NBLK):
                nc.tensor.matmul(out=ps_s[:], lhsT=onesb_sb[:], rhs=h_raw[:, b * OC : (b + 1) * OC], start=(b == 0), stop=(b == NBLK - 1))
                sq = sb.tile([128, OC], dt.bfloat16, tag="sq")
                nc.vector.tensor_tensor(out=sq[:], in0=h_raw[:, b * OC : (b + 1) * OC], in1=h_raw[:, b * OC : (b + 1) * OC], op=ALU.mult)
                nc.tensor.matmul(out=ps_q[:], lhsT=onesb_sb[:], rhs=sq[:], start=(b == 0), stop=(b == NBLK - 1))
            stats = sb.tile([1, 2 * OC], dt.float32, tag="stats", bufs=1)
            nc.vector.tensor_copy(stats[0:1, 0:OC], ps_s[:])
            nc.vector.tensor_copy(stats[0:1, OC : 2 * OC], ps_q[:])
            st_in = dram.tile([1, 2 * OC], dt.float32, tag="st_in")
            st_out = dram.tile([1, 2 * OC], dt.float32, tag="st_out", addr_space="Shared")
            nc.sync.dma_start(st_in[:], stats[:])
            nc.gpsimd.collective_compute(
                "AllReduce", ALU.add, ins=[st_in[:]], outs=[st_out[:]], replica_groups=[list(range(NC))]
            )
            st2 = sb.tile([1, 2 * OC], dt.float32, tag="st2", bufs=1)
            nc.sync.dma_start(st2[:], st_out[:])
            m = sb.tile([1, OC], dt.float32, tag="bn_m", bufs=1)
            q = sb.tile([1, OC], dt.float32, tag="bn_q", bufs=1)
            nc.vector.tensor_scalar(out=m[:], in0=st2[0:1, 0:OC], scalar1=1.0 / N_NODES, scalar2=None, op0=ALU.mult)
            nc.vector.tensor_scalar(out=q[:], in0=st2[0:1, OC : 2 * OC], scalar1=1.0 / N_NODES, scalar2=None, op0=ALU.mult)
            var = sb.tile([1, OC], dt.float32, tag="bn_v", bufs=1)
            nc.vector.tensor_tensor(out=var[:], in0=m[:], in1=m[:], op=ALU.mult)
            nc.vector.tensor_tensor(out=var[:], in0=q[:], in1=var[:], op=ALU.subtract)
            epsc = sb.tile([1, 1], dt.float32, tag="epsc")
            nc.vector.memset(epsc[:], EPS)
            sd = sb.tile([1, OC], dt.float32, tag="bn_sd", bufs=1)
            nc.scalar.activation(sd[:], var[:], ACTF.Sqrt, bias=epsc[0:1, 0:1])
            rs = sb.tile([1, OC], dt.float32, tag="bn_rs", bufs=1)
            nc.vector.reciprocal(rs[:], sd[:])
            s_row = sb.tile([1, OC], dt.float32, tag="bn_s", bufs=1)
            nc.vector.tensor_tensor(out=s_row[:], in0=rs[:], in1=gam_t[:], op=ALU.mult)
            b_row = sb.tile([1, OC], dt.float32, tag="bn_b", bufs=1)
            nc.vector.tensor_tensor(out=b_row[:], in0=m[:], in1=s_row[:], op=ALU.mult)
            nc.vector.tensor_tensor(out=b_row[:], in0=bet_t[:], in1=b_row[:], op=ALU.subtract)
            sbc = sb.tile([128, 2 * nch], dt.float32, tag="sbc")
            for k in range(nch):
                ps_c = ps.tile([128, 1], dt.float32, tag="small")
                nc.tensor.matmul(out=ps_c[:], lhsT=s_row[0:1, k * 128 : (k + 1) * 128], rhs=onesr_sb[0:1, 0:1], start=True, stop=True)
                nc.vector.tensor_copy(sbc[:, k : k + 1], ps_c[:])
                ps_c2 = ps.tile([128, 1], dt.float32, tag="small")
                nc.tensor.matmul(out=ps_c2[:], lhsT=b_row[0:1, k * 128 : (k + 1) * 128], rhs=onesr_sb[0:1, 0:1], start=True, stop=True)
                nc.vector.tensor_copy(sbc[:, nch + k : nch + k + 1], ps_c2[:])
            # transpose + BN + ELU in stripes of 4 blocks (512 cols)
            SW = 4
            for k in range(nch):
                for s0 in range(0, NBLK, SW):
                    sw = min(SW, NBLK - s0)
                    ystr = sb.tile([128, SW * 128], dt.bfloat16, tag="ystr")
                    for bi in range(sw):
                        b = s0 + bi
                        ps_t = ps.tile([128, 128], dt.bfloat16, tag="med")
                        nc.tensor.matmul(
                            out=ps_t[:], lhsT=h_raw[:, b * OC + k * 128 : b * OC + (k + 1) * 128],
                            rhs=idenb_sb[:], is_transpose=True, start=True, stop=True,
                        )
                        nc.vector.tensor_scalar(
                            out=ystr[:, bi * 128 : (bi + 1) * 128], in0=ps_t[:],
                            scalar1=sbc[:, k : k + 1], scalar2=sbc[:, nch + k : nch + k + 1],
                            op0=ALU.mult, op1=ALU.add,
                        )
                    W = sw * 128
                    t1 = sb.tile([128, SW * 128], dt.bfloat16, tag="elu1")
                    nc.vector.tensor_scalar(out=t1[:, 0:W], in0=ystr[:, 0:W], scalar1=0.0, scalar2=None, op0=ALU.min)
                    e1 = sb.tile([128, SW * 128], dt.bfloat16, tag="elu2")
                    nc.scalar.activation(e1[:, 0:W], t1[:, 0:W], ACTF.Exp)
                    r1 = sb.tile([128, SW * 128], dt.bfloat16, tag="elu3")
                    nc.vector.tensor_scalar(out=r1[:, 0:W], in0=ystr[:, 0:W], scalar1=0.0, scalar2=-1.0, op0=ALU.max, op1=ALU.add)
                    nc.vector.tensor_tensor(
                        out=hTe[:, k * N2 + s0 * 128 : k * N2 + s0 * 128 + W],
                        in0=e1[:, 0:W], in1=r1[:, 0:W], op=ALU.add,
                    )

        # =========================================================
        # run the three layers
        # =========================================================
        x_phase(0, hTe, 0, 1)          # uses hT0 stored in hTe rows 0:64
        edge_phase(0)
        bn_elu_phase(0)
        x_phase(1, hTe, 0, 4)
        edge_phase(1)
        bn_elu_phase(1)
        x_phase(2, hTe, 0, 4)
        edge_phase(2)
        bn_elu_phase(2)

        # =========================================================
        # pooling + classifier
        # =========================================================
        meanT = sb1.tile([128, N_GRAPHS], dt.float32, tag="meanT")
        maxT = sb1.tile([128, N_GRAPHS], dt.float32, tag="maxT")
        nc.vector.memset(meanT[:], 0)
        nc.vector.memset(maxT[:], -1e30)
        mask8_sb = load_sb(i_mask8, [128, NC], dt.float32, "mask8")
        mask8n_sb = load_sb(i_mask8n, [128, NC], dt.float32, "mask8n")
        for cc in range(NC):
            scrm = sb.tile([128, N_GRAPHS], dt.float32, tag="scrm")
            scrx = sb.tile([128, N_GRAPHS], dt.float32, tag="scrx")
            nc.vector.memset(scrm[:], 0)
            nc.vector.memset(scrx[:], -1e30)
            for (a, bnd, g, inv) in segs[cc]:
                r1 = sb.tile([128, 1], dt.float32, tag="segr")
                nc.vector.tensor_reduce(out=r1[:], in_=hTe[:, a:bnd], axis=AXX, op=ALU.add)
                nc.vector.tensor_scalar(out=scrm[:, g : g + 1], in0=r1[:], scalar1=inv, scalar2=None, op0=ALU.mult)
                nc.vector.tensor_reduce(out=scrx[:, g : g + 1], in_=hTe[:, a:bnd], axis=AXX, op=ALU.max)
            nc.vector.tensor_scalar(out=scrm[:], in0=scrm[:], scalar1=mask8_sb[:, cc : cc + 1], scalar2=None, op0=ALU.mult)
            nc.vector.tensor_tensor(out=meanT[:], in0=meanT[:], in1=scrm[:], op=ALU.add)
            nc.vector.tensor_scalar(
                out=scrx[:], in0=scrx[:], scalar1=mask8_sb[:, cc : cc + 1],
                scalar2=mask8n_sb[:, cc : cc + 1], op0=ALU.mult, op1=ALU.add,
            )
            nc.vector.tensor_tensor(out=maxT[:], in0=maxT[:], in1=scrx[:], op=ALU.max)
        pm_in = dram.tile([128, N_GRAPHS], dt.float32, tag="pm_in")
        pm_out = dram.tile([128, N_GRAPHS], dt.float32, tag="pm_out", addr_space="Shared")
        px_in = dram.tile([128, N_GRAPHS], dt.float32, tag="px_in")
        px_out = dram.tile([128, N_GRAPHS], dt.float32, tag="px_out", addr_space="Shared")
        nc.sync.dma_start(pm_in[:], meanT[:])
        nc.sync.dma_start(px_in[:], maxT[:])
        nc.gpsimd.collective_compute("AllReduce", ALU.add, ins=[pm_in[:]], outs=[pm_out[:]], replica_groups=[list(range(NC))])
        nc.gpsimd.collective_compute("AllReduce", ALU.max, ins=[px_in[:]], outs=[px_out[:]], replica_groups=[list(range(NC))])
        meanF = sb1.tile([128, N_GRAPHS], dt.float32, tag="meanF")
        maxF = sb1.tile([128, N_GRAPHS], dt.float32, tag="maxF")
        nc.sync.dma_start(meanF[:], pm_out[:])
        nc.sync.dma_start(maxF[:], px_out[:])

        cw1t_sb = [None, None]
        cw2t_sb = [None, None]
        for k in range(2):
            cw1t_sb[k] = sb1.tile([128, GDIM], dt.float32, tag=f"cw1t{k}", name=f"cw1t{k}")
            nc.sync.dma_start(cw1t_sb[k][:], P["cw1t"][k * 128 : (k + 1) * 128, :])
            cw2t_sb[k] = sb1.tile([128, NUM_CLASSES], dt.float32, tag=f"cw2t{k}", name=f"cw2t{k}")
            nc.sync.dma_start(cw2t_sb[k][:], P["cw2t"][k * 128 : (k + 1) * 128, :])
        cb1c_sb = load_sb(P["cb1c"], [128, 2], dt.float32, "cb1c")
        cb2c_sb = load_sb(P["cb2c"], [NUM_CLASSES, 1], dt.float32, "cb2c")
        hidT = sb1.tile([128, 2 * N_GRAPHS], dt.float32, tag="hidT")
        for hc in range(2):
            ps_h = ps.tile([128, N_GRAPHS], dt.float32, tag="med")
            for dc, embT in enumerate([meanF, maxF]):
                nc.tensor.matmul(
                    out=ps_h[:], lhsT=cw1t_sb[dc][:, hc * 128 : (hc + 1) * 128],
                    rhs=embT[:], start=(dc == 0), stop=(dc == 1),
                )
            nc.vector.tensor_scalar(
                out=hidT[:, hc * N_GRAPHS : (hc + 1) * N_GRAPHS], in0=ps_h[:],
                scalar1=cb1c_sb[:, hc : hc + 1], scalar2=0.0, op0=ALU.add, op1=ALU.max,
            )
        ps_o = ps.tile([NUM_CLASSES, N_GRAPHS], dt.float32, tag="med")
        for hc in range(2):
            nc.tensor.matmul(
                out=ps_o[:], lhsT=cw2t_sb[hc][:],
                rhs=hidT[:, hc * N_GRAPHS : (hc + 1) * N_GRAPHS], start=(hc == 0), stop=(hc == 1),
            )
        osb = sb1.tile([NUM_CLASSES, N_GRAPHS], dt.float32, tag="osb")
        nc.vector.tensor_scalar(out=osb[:], in0=ps_o[:], scalar1=cb2c_sb[:], scalar2=None, op0=ALU.add)
        for gc in range(2):
            ps_tt = ps.tile([128, NUM_CLASSES], dt.float32, tag="med")
            nc.tensor.matmul(
                out=ps_tt[:], lhsT=osb[:, gc * 128 : (gc + 1) * 128],
                rhs=idenf_sb[0:NUM_CLASSES, 0:NUM_CLASSES], start=True, stop=True,
            )
            ot = sb1.tile([128, NUM_CLASSES], dt.float32, tag="ot")
            nc.vector.tensor_copy(ot[:], ps_tt[:])
            nc.sync.dma_start(out_dram[gc * 128 : (gc + 1) * 128, :], ot[:])

    nc.compile()
    return nc


def kernel(**inputs):
    import concourse.bass_utils as bass_utils

    pre = preprocess(inputs["x"], inputs["edge_index"], inputs["depth"], inputs["batch"])
    pb = build_param_blobs(inputs)
    import hashlib

    sig = hashlib.sha1(
        pre["TloU"].tobytes() + pre["ThiU"].tobytes() + repr(pre["segs"]).encode()
    ).hexdigest()
    if _CACHE.get("sig") != sig:
        _CACHE["built"] = build_nc(pre)
        _CACHE["sig"] = sig
    nc = _CACHE["built"]

    in_maps = []
    for c in range(NC):
        b = pre["blobs"][c]
        m = dict(
            idx_lo=b["idx_lo"], idx_hi=b["idx_hi"], idx_ald=b["idx_ald"],
            dstloc=b["dstloc"].astype(np.float32), emb_idx=b["emb_idx"],
            depth_row=b["depth_row"], mask8=b["mask8"], mask8n=b["mask8n"],
        )
        m.update({k: v for k, v in pb.items()})
        in_maps.append(m)

    import os, time

    trace = bool(int(os.environ.get("KERNEL_TRACE", "0")))
    t0 = time.time()
    res = bass_utils.run_bass_kernel_spmd(
        nc, in_maps, core_ids=list(range(NC)), trace=trace
    )
    _CACHE["run_s"] = time.time() - t0
    _CACHE["last_results"] = res
    return np.asarray(res.results[0]["out"], dtype=np.float32)


if __name__ == "__main__":
    sys.path.insert(0, "/root/problem")
    import reference

    inp = {k: np.asarray(v) for k, v in reference.setup_inputs().items()}
    got = kernel(**inp)
    exp = np.asarray(reference.reference(**inp))
    err = np.abs(got - exp).max() / (np.abs(exp).max() + 1e-30)
    print("Relative error:", err)
